# revision 21
# baseline (speedup 1.0000x reference)
"""LSTM decoder kernel for Trainium2 (8 NeuronCores, data-parallel over batch).

Reference computation (per batch element b):
    h0 = context_seq[b, -1, :]          # only the LAST timestep is used
    c0 = 0
    for t in range(T):
        gates = h @ (W_ih + W_hh).T + (b_ih + b_hh)     # [4H], order i,f,g,o
        i, f, g, o = split(gates)
        c = sigmoid(f) * c + sigmoid(i) * tanh(g)
        h = sigmoid(o) * tanh(c)
        pred[t] = h @ W_out.T + b_out                   # [O]

Device layout (per core, B=128 batch rows): state kept TRANSPOSED — hT, cT
are [H=128 partitions, B free], so no per-step transposes are needed and
per-partition ACT bias lines up with gate rows. Per-step prediction via a
small matmul (stationary = hT) giving pred [B, 7] naturally, accumulated in
SBUF, one DMA at the end. b_out is added on the host.

Shipped design (VARIANT 13, "fused2", HW 2.28 us/step = 1.17 ms at T=512):
  - bf16 matmuls (weights + h state bf16, PSUM accumulates fp32; c stays
    fp32). End-to-end rel err vs fp32 reference: 3.3e-3 (tolerance 2e-2).
  - tanh eliminated via tanh(x) = 2*sigmoid(2x) - 1. The x2/x4 factors are
    folded into the bf16 weights host-side (exact powers of two; state
    stores h' = h/2), the -0.5/x2 affines ride fused DVE
    scalar_tensor_tensor ops. ACT does TWO sigmoids per stream-step:
    one over all four gate blocks [g|f|i|o], one for tanh(c).
  - Gate biases pre-accumulated into PSUM by a K=4 one-hot bf16 matmul
    with no h dependency (off the critical path). NOTE: PSUM start=True
    zeroes the whole 2KB bank, so each accumulation group owns its bank.
  - Two half-step phase-offset streams of 64 batch columns, emission
    interleaved [A.front, B.tail, B.front, A.tail] so the ACT-bound
    engine alternates between streams and the 8-hop recurrence chain
    latency is hidden.
  - t1 = sigmoid(f)*c on the Pool engine (plain tensor_mul only — fused
    ops on Pool hit the slow Q7 ucode path).

Earlier variants (HW-measured per-step time at T=512, 8 cores):
  1: fp32, per-gate ACT bias, one gates PSUM bank            (4.2 us/step)
  4: fp32, merged sigmoid via K=3 one-hot bias matmul        (5.5 us/step)
  6: variant 1 x two phase-offset streams of B/2             (5.8 us/step)
  7: fp32, gates split {f,i}/{g,o} PSUM banks                (3.4 us/step)
  8: fp32, one PSUM bank per gate                            (4.1 us/step)
  12: bf16, merged sigmoid(f,i,o), tanh kept                 (2.65 us/step)
  13: all-sigmoid fused2 (above)                             (2.28 us/step)
"""

import json

import numpy as np

B_TOTAL = 1024
H = 128
O = 7
N_CORES = 8
B_CORE = B_TOTAL // N_CORES  # 128

VARIANT = 13

ACTS_BUFS = 2
_N_STREAMS = {1: 1, 4: 1, 5: 2, 6: 2, 7: 1, 8: 1, 9: 1, 11: 1, 12: 2, 13: 2, 14: 1}
_STYLE = {
    1: "acts",
    4: "biasmm",
    5: "biasmm",
    6: "acts",
    7: "acts2",
    8: "acts4",
    9: "acts2",
    11: "fused",
    12: "fused",
    13: "fused2",
    14: "fused2",
}


def _split_multiwait(bir_bytes: bytes) -> bytes:
    """This walrus build encodes at most ONE sync-wait per instruction.
    Split any multi-wait instruction into single-wait NoOps on the same
    engine (the sequencer executes them in program order, so waiting on
    each semaphore in turn is equivalent to waiting on all of them)."""
    bir = json.loads(bir_bytes)
    n = 0
    for f in bir.get("functions", []):
        for blk in f.get("blocks", []):
            new = []
            for inst in blk.get("instructions", []):
                si = inst.get("sync_info")
                waits = (si or {}).get("on_wait") or []
                if len(waits) > 1:
                    for w in waits[:-1]:
                        n += 1
                        nop = {
                            "name": f"WSPLIT-{n}",
                            "engine": inst.get("engine"),
                            "ins": [],
                            "outs": [],
                            "opcode": "NoOp",
                            "sync_info": {"on_update": [], "on_wait": [w]},
                        }
                        if inst.get("debug") is not None:
                            nop["debug"] = inst["debug"]
                        new.append(nop)
                    si["on_wait"] = [waits[-1]]
                new.append(inst)
            blk["instructions"] = new
    return json.dumps(bir).encode()


_PATCHED = False


def _patch_bass():
    global _PATCHED
    if _PATCHED:
        return
    import concourse.bass as bass

    orig = bass.Bass.to_json_bytes

    def patched(self, *a, **k):
        return _split_multiwait(orig(self, *a, **k))

    bass.Bass.to_json_bytes = patched
    _PATCHED = True


_PROGRAM_CACHE = {}


class _Stream:
    """Per-stream tiles + emit logic for one LSTM step."""

    gp_t2 = False

    def __init__(self, nc, tc, pools, consts, s, Bs, style):
        from concourse import mybir

        fp32 = mybir.dt.float32
        self.nc = nc
        self.s = s
        self.Bs = Bs
        self.style = style
        self.consts = consts
        self.state, self.acts, self.psum, self.ppsum, self.outp = pools
        self.h = None  # set by caller
        self.c = None
        self.outbuf = self.outp.tile(
            [Bs, consts["T"] * O], fp32, tag=f"outbuf{s}", name=f"outbuf{s}"
        )
        self.pred_pps = None  # 2-step batched pred psum tile

    def step(self, t):
        nc = self.nc
        from concourse import mybir

        fp32 = mybir.dt.float32
        AF = mybir.ActivationFunctionType
        s, Bs = self.s, self.Bs
        C = self.consts
        wt, bias, woutt = C["wt"], C["bias"], C["woutt"]

        if self.style == "biasmm":
            bstack, onehot = C["bstack"], C["onehot"]
            gp = self.psum.tile([128, 4 * Bs], fp32, tag=f"g{s}", bufs=2)
            # bias init for i,f,o cols [0:3Bs) — independent of h, prefetchable
            nc.tensor.matmul(gp[:, 0 : 3 * Bs], bstack[:], onehot[:],
                             start=True, stop=False, skip_group_check=True)
            for g in range(4):  # gate order in wt: i,f,o,g
                nc.tensor.matmul(
                    gp[:, g * Bs : (g + 1) * Bs],
                    wt[:, g * H : (g + 1) * H],
                    self.h[:],
                    start=False,
                    stop=(g == 3),
                    skip_group_check=True,
                )
            ifo = self.acts.tile([H, 3 * Bs], fp32, tag=f"ifo{s}", name=f"ifo{s}")
            nc.scalar.activation(ifo[:], gp[:, 0 : 3 * Bs], AF.Sigmoid)
            g_t = self.acts.tile([H, Bs], fp32, tag=f"gt{s}", name=f"gt{s}")
            nc.scalar.activation(g_t[:], gp[:, 3 * Bs : 4 * Bs], AF.Tanh,
                                 bias=bias[:, 3:4])
            i_s = ifo[:, 0:Bs]
            f_s = ifo[:, Bs : 2 * Bs]
            o_s = ifo[:, 2 * Bs : 3 * Bs]
        elif self.style == "acts4":
            # one PSUM bank per gate: each sigmoid starts right after its own
            # matmul; wt order i,f,g,o; ACT order f,i,g,o
            banks = {}
            for g, tag, bufs in ((1, "bf", 2), (0, "bi", 2), (2, "bg", 1), (3, "bo", 1)):
                pb = self.psum.tile([128, Bs], fp32, tag=f"{tag}{s}", bufs=bufs)
                nc.tensor.matmul(pb[:], wt[:, g * H : (g + 1) * H], self.h[:],
                                 start=True, stop=True)
                banks[g] = pb
            f_t = self.acts.tile([H, Bs], fp32, tag=f"fs{s}", name=f"fs{s}")
            nc.scalar.activation(f_t[:], banks[1][:], AF.Sigmoid, bias=bias[:, 1:2])
            i_t = self.acts.tile([H, Bs], fp32, tag=f"is{s}", name=f"is{s}")
            nc.scalar.activation(i_t[:], banks[0][:], AF.Sigmoid, bias=bias[:, 0:1])
            g_t = self.acts.tile([H, Bs], fp32, tag=f"gt{s}", name=f"gt{s}")
            nc.scalar.activation(g_t[:], banks[2][:], AF.Tanh, bias=bias[:, 2:3])
            o_t = self.acts.tile([H, Bs], fp32, tag=f"os{s}", name=f"os{s}")
            nc.scalar.activation(o_t[:], banks[3][:], AF.Sigmoid, bias=bias[:, 3:4])
            i_s, f_s, g_t, o_s = i_t[:], f_t[:], g_t, o_t[:]
        elif self.style == "acts2":
            # per-gate ACT bias, but gates split across TWO PSUM banks
            # ({f,i} and {g,o}) so sigmoid(f) starts after two matmuls
            # instead of four; wt order i,f,g,o
            gfi = self.psum.tile([128, 2 * Bs], fp32, tag=f"gfi{s}", bufs=2)
            ggo = self.psum.tile([128, 2 * Bs], fp32, tag=f"ggo{s}", bufs=1)
            for g, dst, col in ((1, gfi, 0), (0, gfi, 1), (2, ggo, 0), (3, ggo, 1)):
                nc.tensor.matmul(
                    dst[:, col * Bs : (col + 1) * Bs],
                    wt[:, g * H : (g + 1) * H],
                    self.h[:],
                    start=True,
                    stop=True,
                )
            f_t = self.acts.tile([H, Bs], fp32, tag=f"fs{s}", name=f"fs{s}")
            nc.scalar.activation(f_t[:], gfi[:, 0:Bs], AF.Sigmoid, bias=bias[:, 1:2])
            i_t = self.acts.tile([H, Bs], fp32, tag=f"is{s}", name=f"is{s}")
            nc.scalar.activation(i_t[:], gfi[:, Bs : 2 * Bs], AF.Sigmoid, bias=bias[:, 0:1])
            g_t = self.acts.tile([H, Bs], fp32, tag=f"gt{s}", name=f"gt{s}")
            nc.scalar.activation(g_t[:], ggo[:, 0:Bs], AF.Tanh, bias=bias[:, 2:3])
            o_t = self.acts.tile([H, Bs], fp32, tag=f"os{s}", name=f"os{s}")
            nc.scalar.activation(o_t[:], ggo[:, Bs : 2 * Bs], AF.Sigmoid, bias=bias[:, 3:4])
            i_s, f_s, g_t, o_s = i_t[:], f_t[:], g_t, o_t[:]
        else:  # "acts": per-gate ACT with per-partition bias; wt order i,f,g,o
            gp = self.psum.tile([128, 4 * Bs], fp32, tag=f"g{s}", bufs=2)
            for g in (1, 0, 2, 3):  # emit f first: t1 depends on f alone
                nc.tensor.matmul(
                    gp[:, g * Bs : (g + 1) * Bs],
                    wt[:, g * H : (g + 1) * H],
                    self.h[:],
                    start=True,
                    stop=True,
                )
            f_t = self.acts.tile([H, Bs], fp32, tag=f"fs{s}", name=f"fs{s}")
            nc.scalar.activation(f_t[:], gp[:, Bs : 2 * Bs], AF.Sigmoid, bias=bias[:, 1:2])
            i_t = self.acts.tile([H, Bs], fp32, tag=f"is{s}", name=f"is{s}")
            nc.scalar.activation(i_t[:], gp[:, 0:Bs], AF.Sigmoid, bias=bias[:, 0:1])
            g_t = self.acts.tile([H, Bs], fp32, tag=f"gt{s}", name=f"gt{s}")
            nc.scalar.activation(g_t[:], gp[:, 2 * Bs : 3 * Bs], AF.Tanh, bias=bias[:, 2:3])
            o_t = self.acts.tile([H, Bs], fp32, tag=f"os{s}", name=f"os{s}")
            nc.scalar.activation(o_t[:], gp[:, 3 * Bs : 4 * Bs], AF.Sigmoid, bias=bias[:, 3:4])
            i_s, f_s, g_t, o_s = i_t[:], f_t[:], g_t, o_t[:]

        t1 = self.acts.tile([H, Bs], fp32, tag=f"t1{s}", name=f"t1{s}")
        nc.vector.tensor_mul(t1[:], f_s, self.c[:])
        t2 = self.acts.tile([H, Bs], fp32, tag=f"t2{s}", name=f"t2{s}")
        if self.gp_t2:
            nc.gpsimd.tensor_mul(t2[:], i_s, g_t[:])
        else:
            nc.vector.tensor_mul(t2[:], i_s, g_t[:])
        c_new = self.state.tile([H, Bs], fp32, tag=f"c{s}", name=f"c{s}")
        nc.vector.tensor_add(c_new[:], t1[:], t2[:])
        th = self.acts.tile([H, Bs], fp32, tag=f"th{s}", name=f"th{s}")
        nc.scalar.activation(th[:], c_new[:], AF.Tanh)
        h_new = self.state.tile([H, Bs], fp32, tag=f"h{s}", name=f"h{s}")
        nc.vector.tensor_mul(h_new[:], o_s, th[:])
        self.h, self.c = h_new, c_new

        # prediction: out [Bs, O] = h_new.T @ woutt; batch PB steps per PSUM
        # tile + one DVE copy (an accumulation group writing disjoint slots)
        PB = 4 if self.style in ("acts2", "acts4") else 2
        k = t % PB
        if k == 0:
            self.pred_pps = self.ppsum.tile([Bs, PB * O], fp32, tag=f"pp{s}", bufs=2)
        nc.tensor.matmul(self.pred_pps[:, k * O : (k + 1) * O], h_new[:], woutt[:],
                         start=(k == 0), stop=(k == PB - 1), skip_group_check=True)
        if k == PB - 1 or t == self.consts["T"] - 1:
            nc.vector.tensor_copy(
                self.outbuf[:, (t - k) * O : (t + 1) * O],
                self.pred_pps[:, 0 : (k + 1) * O],
            )


class _FusedStream:
    """bf16-matmul fused-activation stream.

    Weight column order [g|f|i|o]: tanh(g) fires after the first matmul;
    sigmoid is ONE merged ACT over the contiguous f,i,o block whose biases
    are pre-accumulated into PSUM by a rank-3 one-hot bf16 matmul that does
    not depend on h (off the critical path). g's bias rides the ACT bias
    operand. h is kept in bf16 (weights/matmuls bf16, c stays fp32).
    """

    PB = 8  # pred steps batched per PSUM tile
    T2_GPSIMD = True  # t2 = sig(i)*tanh(g) on Pool engine (else DVE)
    HMUL_GPSIMD = False  # h = sig(o)*tanh(c) on Pool engine (else DVE)
    NO_PRED = False  # diagnostic: skip the per-step prediction matmul
    ORDER_ALT = False  # 2-stream emission: A.front,B.tail,A.tail,B.front

    def __init__(self, nc, pools, consts, s, Bs):
        from concourse import mybir

        fp32 = mybir.dt.float32
        self.nc = nc
        self.s = s
        self.Bs = Bs
        self.consts = consts
        self.state, self.acts, self.psum, self.ppsum, self.outp = pools
        self.h = None
        self.c = None
        self.outbuf = self.outp.tile(
            [Bs, consts["T"] * O], fp32, tag=f"outbuf{s}", name=f"outbuf{s}"
        )
        self.pred_pps = None

    def front(self, t):
        nc = self.nc
        from concourse import mybir

        fp32 = mybir.dt.float32
        AF = mybir.ActivationFunctionType
        s, Bs = self.s, self.Bs
        C = self.consts
        wt, bfio, onehot3, biasg = C["wt"], C["bfio"], C["onehot3"], C["biasg"]

        # PSUM start=True zeroes the whole 2KB bank (ZERO_REGION), so the
        # bias-accumulation group {bias, f, i, o} must own its bank; the g
        # gate (own start=True) lives in a separate bank.
        gfio = self.psum.tile([128, 3 * Bs], fp32, tag=f"gfio{s}", bufs=2)
        # f,i,o bias pre-accumulate: no h dependency, prefetchable
        nc.tensor.matmul(gfio[:], bfio[:], onehot3[:],
                         start=True, stop=False, skip_group_check=True)
        # g first so tanh(g) can start after one matmul
        gg = self.psum.tile([128, Bs], fp32, tag=f"gg{s}", bufs=1)
        nc.tensor.matmul(gg[:], wt[:, 0:H], self.h[:],
                         start=True, stop=True, skip_group_check=True)
        for j, g in enumerate((1, 2, 3)):  # f, i, o
            nc.tensor.matmul(
                gfio[:, j * Bs : (j + 1) * Bs],
                wt[:, g * H : (g + 1) * H],
                self.h[:],
                start=False,
                stop=(j == 2),
                skip_group_check=True,
            )
        g_t = self.acts.tile([H, Bs], fp32, tag=f"gt{s}", name=f"gt{s}")
        nc.scalar.activation(g_t[:], gg[:], AF.Tanh, bias=biasg[:, 0:1])
        sfio = self.acts.tile([H, 3 * Bs], fp32, tag=f"sfio{s}", name=f"sfio{s}")
        nc.scalar.activation(sfio[:], gfio[:], AF.Sigmoid)
        self.g_t, self.sfio = g_t, sfio

    def tail(self, t):
        nc = self.nc
        from concourse import mybir

        fp32 = mybir.dt.float32
        bf16 = mybir.dt.bfloat16
        AF = mybir.ActivationFunctionType
        s, Bs = self.s, self.Bs
        g_t, sfio = self.g_t, self.sfio
        woutt = self.consts["woutt"]

        t1 = self.acts.tile([H, Bs], fp32, tag=f"t1{s}", name=f"t1{s}")
        nc.vector.tensor_mul(t1[:], sfio[:, 0:Bs], self.c[:])
        t2 = self.acts.tile([H, Bs], fp32, tag=f"t2{s}", name=f"t2{s}")
        t2_eng = nc.gpsimd if self.T2_GPSIMD else nc.vector
        t2_eng.tensor_mul(t2[:], sfio[:, Bs : 2 * Bs], g_t[:])
        c_new = self.state.tile([H, Bs], fp32, tag=f"c{s}", name=f"c{s}")
        nc.vector.tensor_add(c_new[:], t1[:], t2[:])
        th = self.acts.tile([H, Bs], fp32, tag=f"th{s}", name=f"th{s}")
        nc.scalar.activation(th[:], c_new[:], AF.Tanh)
        h_new = self.state.tile([H, Bs], bf16, tag=f"h{s}", name=f"h{s}")
        hm_eng = nc.gpsimd if self.HMUL_GPSIMD else nc.vector
        hm_eng.tensor_mul(h_new[:], sfio[:, 2 * Bs : 3 * Bs], th[:])
        self.h, self.c = h_new, c_new

        if self.NO_PRED:
            return
        PB = self.PB
        k = t % PB
        if k == 0:
            self.pred_pps = self.ppsum.tile([Bs, PB * O], fp32, tag=f"pp{s}", bufs=1)
        nc.tensor.matmul(self.pred_pps[:, k * O : (k + 1) * O], h_new[:], woutt[:],
                         start=(k == 0), stop=(k == PB - 1), skip_group_check=True)
        if k == PB - 1 or t == self.consts["T"] - 1:
            nc.vector.tensor_copy(
                self.outbuf[:, (t - k) * O : (t + 1) * O],
                self.pred_pps[:, 0 : (k + 1) * O],
            )


class _Fused2Stream(_FusedStream):
    """All-sigmoid variant: tanh(x) = 2*sigmoid(2x) - 1.

    State h is stored HALVED (h' = h/2); compensating factors are folded
    into the bf16 weights host-side (exact powers of two): gate weights x2,
    g-gate x4 total, W_out x2, g bias x2. ONE sigmoid covers all four gate
    blocks [g|f|i|o] (biases pre-accumulated in PSUM via a K=4 one-hot
    matmul); tanh(c) is sigmoid(2c) plus a fused affine in the h' update:
      t2h = (sg2 - 0.5) * si          (= i*tanh(g)/2)
      c'  = (t2h * 2) + t1            (t1 = sf * c)
      h'  = (sc2 - 0.5) * so          (= h/2)
    ACT does 2 sigmoids per stream-step instead of 3 acts.
    """

    def front(self, t):
        nc = self.nc
        from concourse import mybir

        fp32 = mybir.dt.float32
        AF = mybir.ActivationFunctionType
        s, Bs = self.s, self.Bs
        C = self.consts
        wt, bstack4, onehot4 = C["wt"], C["bstack4"], C["onehot4"]

        gp = self.psum.tile([128, 4 * Bs], fp32, tag=f"g{s}", bufs=2)
        # all-four-gate bias pre-accumulate: no h dependency, prefetchable
        nc.tensor.matmul(gp[:], bstack4[:], onehot4[:],
                         start=True, stop=False, skip_group_check=True)
        for j in range(4):  # g, f, i, o
            nc.tensor.matmul(
                gp[:, j * Bs : (j + 1) * Bs],
                wt[:, j * H : (j + 1) * H],
                self.h[:],
                start=False,
                stop=(j == 3),
                skip_group_check=True,
            )
        sg = self.acts.tile([H, 4 * Bs], fp32, tag=f"sg{s}", name=f"sg{s}")
        nc.scalar.activation(sg[:], gp[:], AF.Sigmoid)
        self.sg = sg

    def tail(self, t):
        nc = self.nc
        from concourse import mybir

        fp32 = mybir.dt.float32
        bf16 = mybir.dt.bfloat16
        AF = mybir.ActivationFunctionType
        Op = mybir.AluOpType
        s, Bs = self.s, self.Bs
        sg = self.sg
        woutt = self.consts["woutt"]
        sg2 = sg[:, 0:Bs]
        sf = sg[:, Bs : 2 * Bs]
        si = sg[:, 2 * Bs : 3 * Bs]
        so = sg[:, 3 * Bs : 4 * Bs]

        # plain tensor_mul is the fast path on Pool; the fused
        # scalar_tensor_tensor ops stay on DVE
        t1 = self.acts.tile([H, Bs], fp32, tag=f"t1{s}", name=f"t1{s}")
        t1_eng = nc.gpsimd if self.T2_GPSIMD else nc.vector
        t1_eng.tensor_mul(t1[:], sf, self.c[:])
        t2h = self.acts.tile([H, Bs], fp32, tag=f"t2{s}", name=f"t2{s}")
        nc.vector.scalar_tensor_tensor(t2h[:], sg2, 0.5, si, Op.subtract, Op.mult)
        c_new = self.state.tile([H, Bs], fp32, tag=f"c{s}", name=f"c{s}")
        nc.vector.scalar_tensor_tensor(c_new[:], t2h[:], 2.0, t1[:], Op.mult, Op.add)
        sc = self.acts.tile([H, Bs], fp32, tag=f"th{s}", name=f"th{s}")
        nc.scalar.activation(sc[:], c_new[:], AF.Sigmoid, scale=2.0)
        h_new = self.state.tile([H, Bs], bf16, tag=f"h{s}", name=f"h{s}")
        nc.vector.scalar_tensor_tensor(h_new[:], sc[:], 0.5, so, Op.subtract, Op.mult)
        self.h, self.c = h_new, c_new

        if self.NO_PRED:
            return
        PB = self.PB
        k = t % PB
        if k == 0:
            self.pred_pps = self.ppsum.tile([Bs, PB * O], fp32, tag=f"pp{s}", bufs=2)
        nc.tensor.matmul(self.pred_pps[:, k * O : (k + 1) * O], h_new[:], woutt[:],
                         start=(k == 0), stop=(k == PB - 1), skip_group_check=True)
        if k == PB - 1 or t == self.consts["T"] - 1:
            nc.vector.tensor_copy(
                self.outbuf[:, (t - k) * O : (t + 1) * O],
                self.pred_pps[:, 0 : (k + 1) * O],
            )


def _build_program_fused(T: int, n_streams: int, repeat: int = 1, style: str = "fused"):
    import concourse.bass as bass
    import concourse.tile as tile
    from concourse import mybir

    _patch_bass()

    fp32 = mybir.dt.float32
    bf16 = mybir.dt.bfloat16
    Bs = B_CORE // n_streams

    nc = bass.Bass("TRN2", debug=False)
    d_h0t = nc.dram_tensor("h0t", [H, B_CORE], bf16, kind="ExternalInput").ap()
    d_wt = nc.dram_tensor("wt", [H, 4 * H], bf16, kind="ExternalInput").ap()
    if style == "fused2":
        d_bstack4 = nc.dram_tensor("bstack4", [4, H], bf16, kind="ExternalInput").ap()
        d_onehot4 = nc.dram_tensor(
            "onehot4", [4, 4 * Bs], bf16, kind="ExternalInput"
        ).ap()
    else:
        d_bfio = nc.dram_tensor("bfio", [3, H], bf16, kind="ExternalInput").ap()
        d_onehot3 = nc.dram_tensor(
            "onehot3", [3, 3 * Bs], bf16, kind="ExternalInput"
        ).ap()
        d_biasg = nc.dram_tensor("biasg", [H, 1], fp32, kind="ExternalInput").ap()
    d_woutt = nc.dram_tensor("woutt", [H, O], bf16, kind="ExternalInput").ap()
    d_preds = nc.dram_tensor("preds", [B_CORE, T * O], fp32, kind="ExternalOutput").ap()

    with tile.TileContext(nc) as tc:
        with (
            tc.tile_pool(name="fixed", bufs=1) as fixed,
            tc.tile_pool(name="state", bufs=2) as state,
            tc.tile_pool(name="acts", bufs=ACTS_BUFS) as acts,
            tc.tile_pool(name="psum", bufs=2, space="PSUM") as psum_pool,
            tc.tile_pool(name="ppsum", bufs=2, space="PSUM") as ppsum_pool,
            tc.tile_pool(name="outp", bufs=1) as outp,
        ):
            consts = {"T": T}
            wt = fixed.tile([H, 4 * H], bf16)
            nc.sync.dma_start(wt[:], d_wt[:])
            if style == "fused2":
                bstack4 = fixed.tile([4, H], bf16)
                nc.sync.dma_start(bstack4[:], d_bstack4[:])
                onehot4 = fixed.tile([4, 4 * Bs], bf16)
                nc.sync.dma_start(onehot4[:], d_onehot4[:])
                consts.update(bstack4=bstack4, onehot4=onehot4)
            else:
                bfio = fixed.tile([3, H], bf16)
                nc.sync.dma_start(bfio[:], d_bfio[:])
                onehot3 = fixed.tile([3, 3 * Bs], bf16)
                nc.sync.dma_start(onehot3[:], d_onehot3[:])
                biasg = fixed.tile([H, 1], fp32)
                nc.sync.dma_start(biasg[:], d_biasg[:])
                consts.update(bfio=bfio, onehot3=onehot3, biasg=biasg)
            woutt = fixed.tile([H, O], bf16)
            nc.sync.dma_start(woutt[:], d_woutt[:])
            consts.update(wt=wt, woutt=woutt)

            pools = (state, acts, psum_pool, ppsum_pool, outp)
            cls = _Fused2Stream if style == "fused2" else _FusedStream
            streams = [cls(nc, pools, consts, s, Bs) for s in range(n_streams)]
            h0s = []
            c0s = []
            for s, st in enumerate(streams):
                h0 = state.tile([H, Bs], bf16, tag=f"h{s}", name=f"h0_{s}")
                nc.sync.dma_start(h0[:], d_h0t[:, s * Bs : (s + 1) * Bs])
                c0 = state.tile([H, Bs], fp32, tag=f"c{s}", name=f"c0_{s}")
                nc.vector.memset(c0[:], 0.0)
                st.h, st.c = h0, c0
                h0s.append(h0)
                c0s.append(c0)

            def body():
                if len(streams) == 2:
                    A, B = streams
                    if _FusedStream.ORDER_ALT:
                        for t in range(T):
                            A.front(t)
                            if t > 0:
                                B.tail(t - 1)
                            A.tail(t)
                            B.front(t)
                        B.tail(T - 1)
                    else:
                        for t in range(T):
                            A.front(t)
                            if t > 0:
                                B.tail(t - 1)
                            B.front(t)
                            A.tail(t)
                        B.tail(T - 1)
                else:
                    for t in range(T):
                        for st in streams:
                            st.front(t)
                            st.tail(t)

            if repeat > 1:
                with tc.For_i(0, repeat, 1):
                    body()
                    for s, st in enumerate(streams):
                        nc.vector.tensor_copy(h0s[s][:], st.h[:])
                        nc.vector.tensor_copy(c0s[s][:], st.c[:])
                        st.h, st.c = h0s[s], c0s[s]
            else:
                body()

            for s, st in enumerate(streams):
                nc.sync.dma_start(d_preds[s * Bs : (s + 1) * Bs, :], st.outbuf[:])

    return nc


def _build_program(T: int, variant: int = None, repeat: int = 1):
    if variant is None:
        variant = VARIANT
    if _STYLE[variant] in ("fused", "fused2"):
        return _build_program_fused(T, _N_STREAMS[variant], repeat, _STYLE[variant])
    import concourse.bass as bass
    import concourse.tile as tile
    from concourse import mybir

    _patch_bass()

    fp32 = mybir.dt.float32
    n_streams = _N_STREAMS[variant]
    style = _STYLE[variant]
    Bs = B_CORE // n_streams

    nc = bass.Bass("TRN2", debug=False)
    d_h0t = nc.dram_tensor("h0t", [H, B_CORE], fp32, kind="ExternalInput").ap()
    d_wt = nc.dram_tensor("wt", [H, 4 * H], fp32, kind="ExternalInput").ap()
    d_bias = nc.dram_tensor("bias", [H, 4], fp32, kind="ExternalInput").ap()
    d_woutt = nc.dram_tensor("woutt", [H, O], fp32, kind="ExternalInput").ap()
    if style == "biasmm":
        d_onehot = nc.dram_tensor("onehot", [3, 3 * Bs], fp32, kind="ExternalInput").ap()
    d_preds = nc.dram_tensor("preds", [B_CORE, T * O], fp32, kind="ExternalOutput").ap()

    with tile.TileContext(nc) as tc:
        with (
            tc.tile_pool(name="fixed", bufs=1) as fixed,
            tc.tile_pool(name="state", bufs=2) as state,
            tc.tile_pool(name="acts", bufs=ACTS_BUFS) as acts,
            tc.tile_pool(name="psum", bufs=2, space="PSUM") as psum_pool,
            tc.tile_pool(name="ppsum", bufs=2, space="PSUM") as ppsum_pool,
            tc.tile_pool(name="outp", bufs=1) as outp,
        ):
            consts = {"T": T}
            wt = fixed.tile([H, 4 * H], fp32)
            nc.sync.dma_start(wt[:], d_wt[:])
            bias = fixed.tile([H, 4], fp32)
            nc.sync.dma_start(bias[:], d_bias[:])
            woutt = fixed.tile([H, O], fp32)
            nc.sync.dma_start(woutt[:], d_woutt[:])
            consts.update(wt=wt, bias=bias, woutt=woutt)
            if style == "biasmm":
                bstack = fixed.tile([3, H], fp32)
                nc.sync.dma_start(bstack[:], d_bias.rearrange("h g -> g h")[0:3, :])
                onehot = fixed.tile([3, 3 * Bs], fp32)
                nc.sync.dma_start(onehot[:], d_onehot[:])
                consts.update(bstack=bstack, onehot=onehot)

            pools = (state, acts, psum_pool, ppsum_pool, outp)
            _Stream.gp_t2 = variant == 9
            streams = [
                _Stream(nc, tc, pools, consts, s, Bs, style) for s in range(n_streams)
            ]
            # initial state
            h0s = []
            c0s = []
            for s, st in enumerate(streams):
                h0 = state.tile([H, Bs], fp32, tag=f"h{s}", name=f"h0_{s}")
                nc.sync.dma_start(h0[:], d_h0t[:, s * Bs : (s + 1) * Bs])
                c0 = state.tile([H, Bs], fp32, tag=f"c{s}", name=f"c0_{s}")
                nc.vector.memset(c0[:], 0.0)
                st.h, st.c = h0, c0
                h0s.append(h0)
                c0s.append(c0)

            def body():
                for t in range(T):
                    for st in streams:
                        st.step(t)

            if repeat > 1:
                with tc.For_i(0, repeat, 1):
                    body()
                    for s, st in enumerate(streams):
                        nc.vector.tensor_copy(h0s[s][:], st.h[:])
                        nc.vector.tensor_copy(c0s[s][:], st.c[:])
                        st.h, st.c = h0s[s], c0s[s]
            else:
                body()

            for s, st in enumerate(streams):
                nc.sync.dma_start(d_preds[s * Bs : (s + 1) * Bs, :], st.outbuf[:])

    return nc


_RUNNER_CACHE = {}


def _get_runner(nc):
    """Build (once per program) a jitted shard_map callable over the 8 cores.
    run_bass_kernel_spmd rebuilds its jit closure every call, which retraces
    and re-lowers (including BIR serialization) each time — ~1-2.5s of
    client-side overhead per invocation. Caching the jitted callable makes
    repeat invocations cheap."""
    key = id(nc)
    if key in _RUNNER_CACHE:
        return _RUNNER_CACHE[key]

    import jax
    import numpy as np_
    from jax.sharding import Mesh, PartitionSpec
    from jax.experimental.shard_map import shard_map
    import concourse.mybir as mybir
    from concourse.bass2jax import (
        _bass_exec_p,
        install_neuronx_cc_hook,
        partition_id_tensor,
    )

    install_neuronx_cc_hook()

    partition_name = nc.partition_id_tensor.name if nc.partition_id_tensor else None
    in_names = []
    out_names = []
    out_avals = []
    zero_shapes = []
    for alloc in nc.m.functions[0].allocations:
        if not isinstance(alloc, mybir.MemoryLocationSet):
            continue
        name = alloc.memorylocations[0].name
        if alloc.kind == "ExternalInput":
            if name != partition_name:
                in_names.append(name)
        elif alloc.kind == "ExternalOutput":
            shape = tuple(alloc.tensor_shape)
            dtype = mybir.dt.np(alloc.dtype)
            out_names.append(name)
            out_avals.append(jax.core.ShapedArray(shape, dtype))
            zero_shapes.append((shape, dtype))
    n_params = len(in_names)
    n_outs = len(out_names)
    all_in_names = list(in_names) + list(out_names)
    if partition_name is not None:
        all_in_names.append(partition_name)

    def _body(*args):
        operands = list(args)
        if partition_name is not None:
            operands.append(partition_id_tensor())
        outs = _bass_exec_p.bind(
            *operands,
            out_avals=tuple(out_avals),
            in_names=tuple(all_in_names),
            out_names=tuple(out_names),
            lowering_input_output_aliases=(),
            sim_require_finite=True,
            sim_require_nnan=True,
            nc=nc,
        )
        return tuple(outs)

    donate = tuple(range(n_params, n_params + n_outs))
    devices = jax.devices()[:N_CORES]
    mesh = Mesh(np_.asarray(devices), ("core",))
    in_specs = (PartitionSpec("core"),) * (n_params + n_outs)
    out_specs = (PartitionSpec("core"),) * n_outs
    sharded = jax.jit(
        shard_map(_body, mesh=mesh, in_specs=in_specs, out_specs=out_specs, check_rep=False),
        donate_argnums=donate,
        keep_unused=True,
    )

    def run(in_maps):
        per_core = [[np.asarray(m[name]) for name in in_names] for m in in_maps]
        concat_in = [
            np.concatenate([per_core[c][i] for c in range(N_CORES)], axis=0)
            for i in range(n_params)
        ]
        concat_zeros = [np.zeros((N_CORES * s[0], *s[1:]), d) for s, d in zero_shapes]
        out_arrs = sharded(*concat_in, *concat_zeros)
        return [
            {
                name: np.asarray(out_arrs[i]).reshape(N_CORES, *out_avals[i].shape)[c]
                for i, name in enumerate(out_names)
            }
            for c in range(N_CORES)
        ]

    _RUNNER_CACHE[key] = run
    return run


def _onehot_input(variant):
    if _STYLE[variant] != "biasmm":
        return None
    Bs = B_CORE // _N_STREAMS[variant]
    oh = np.zeros((3, 3 * Bs), dtype=np.float32)
    for g in range(3):
        oh[g, g * Bs : (g + 1) * Bs] = 1.0
    return oh


def _gate_order(variant):
    # order of gate blocks in the wt layout
    return [0, 1, 3, 2] if _STYLE[variant] == "biasmm" else [0, 1, 2, 3]


def _get_program(T: int):
    key = (T, VARIANT)
    if key not in _PROGRAM_CACHE:
        _PROGRAM_CACHE[key] = _build_program(T)
    return _PROGRAM_CACHE[key]


def _bench_in_maps():
    """Random per-core input maps matching the current VARIANT's dram layout."""
    rng = np.random.default_rng(0)
    if _STYLE[VARIANT] == "fused":
        import ml_dtypes

        bf16 = ml_dtypes.bfloat16
        Bs = B_CORE // _N_STREAMS[VARIANT]
        onehot3 = np.zeros((3, 3 * Bs), dtype=bf16)
        for g in range(3):
            onehot3[g, g * Bs : (g + 1) * Bs] = 1.0
        m = {
            "h0t": rng.standard_normal((H, B_CORE)).astype(bf16),
            "wt": (rng.standard_normal((H, 4 * H)) / np.sqrt(H)).astype(bf16),
            "bfio": (rng.standard_normal((3, H)) / np.sqrt(H)).astype(bf16),
            "onehot3": onehot3,
            "biasg": (rng.standard_normal((H, 1)) / np.sqrt(H)).astype(np.float32),
            "woutt": (rng.standard_normal((H, O)) / np.sqrt(H)).astype(bf16),
        }
    elif _STYLE[VARIANT] == "fused2":
        import ml_dtypes

        bf16 = ml_dtypes.bfloat16
        Bs = B_CORE // _N_STREAMS[VARIANT]
        onehot4 = np.zeros((4, 4 * Bs), dtype=bf16)
        for g in range(4):
            onehot4[g, g * Bs : (g + 1) * Bs] = 1.0
        m = {
            "h0t": (0.5 * rng.standard_normal((H, B_CORE))).astype(bf16),
            "wt": (rng.standard_normal((H, 4 * H)) / np.sqrt(H)).astype(bf16),
            "bstack4": (rng.standard_normal((4, H)) / np.sqrt(H)).astype(bf16),
            "onehot4": onehot4,
            "woutt": (rng.standard_normal((H, O)) / np.sqrt(H)).astype(bf16),
        }
    else:
        m = {
            "h0t": rng.standard_normal((H, B_CORE)).astype(np.float32),
            "wt": (rng.standard_normal((H, 4 * H)) / np.sqrt(H)).astype(np.float32),
            "bias": (rng.standard_normal((H, 4)) / np.sqrt(H)).astype(np.float32),
            "woutt": (rng.standard_normal((H, O)) / np.sqrt(H)).astype(np.float32),
        }
        oh = _onehot_input(VARIANT)
        if oh is not None:
            m["onehot"] = oh
    return [dict(m) for _ in range(N_CORES)]


def kernel(
    context_seq,
    W_ih,
    W_hh,
    b_ih,
    b_hh,
    W_out,
    b_out,
    prediction_len,
):
    T = int(prediction_len)
    context_seq = np.asarray(context_seq, dtype=np.float32)
    W_ih = np.asarray(W_ih, dtype=np.float32)
    W_hh = np.asarray(W_hh, dtype=np.float32)
    b_ih = np.asarray(b_ih, dtype=np.float32)
    b_hh = np.asarray(b_hh, dtype=np.float32)
    W_out = np.asarray(W_out, dtype=np.float32)
    b_out = np.asarray(b_out, dtype=np.float32)

    B = context_seq.shape[0]
    assert B == B_TOTAL and context_seq.shape[2] == H

    # Host-side prep: only the last timestep of context_seq is used.
    h0 = context_seq[:, -1, :]  # [B, H]
    W = W_ih + W_hh  # [4H, H]
    b = b_ih + b_hh  # [4H]

    nc = _get_program(T)

    if _STYLE[VARIANT] == "fused":
        import ml_dtypes

        bf16 = ml_dtypes.bfloat16
        Bs = B_CORE // _N_STREAMS[VARIANT]
        order = [2, 1, 0, 3]  # column blocks g|f|i|o from i,f,g,o
        Wb = W.reshape(4, H, H)[order]
        bb = b.reshape(4, H)[order]
        wt = np.ascontiguousarray(Wb.reshape(4 * H, H).T).astype(bf16)
        bfio = np.ascontiguousarray(bb[1:4]).astype(bf16)  # [3, H] f,i,o
        biasg = np.ascontiguousarray(bb[0].reshape(H, 1))  # fp32
        onehot3 = np.zeros((3, 3 * Bs), dtype=bf16)
        for g in range(3):
            onehot3[g, g * Bs : (g + 1) * Bs] = 1.0
        woutt = np.ascontiguousarray(W_out.T).astype(bf16)
        in_maps = []
        for c in range(N_CORES):
            sh = h0[c * B_CORE : (c + 1) * B_CORE]  # [B_CORE, H]
            in_maps.append(
                {
                    "h0t": np.ascontiguousarray(sh.T).astype(bf16),
                    "wt": wt,
                    "bfio": bfio,
                    "onehot3": onehot3,
                    "biasg": biasg,
                    "woutt": woutt,
                }
            )
    elif _STYLE[VARIANT] == "fused2":
        import ml_dtypes

        bf16 = ml_dtypes.bfloat16
        Bs = B_CORE // _N_STREAMS[VARIANT]
        order = [2, 1, 0, 3]  # column blocks g|f|i|o from i,f,g,o
        Wb = W.reshape(4, H, H)[order] * 2.0  # h stored halved
        Wb[0] *= 2.0  # tanh(g) via 2*sigmoid(2g)-1
        bb = b.reshape(4, H)[order].copy()
        bb[0] *= 2.0
        wt = np.ascontiguousarray(Wb.reshape(4 * H, H).T).astype(bf16)
        bstack4 = np.ascontiguousarray(bb).astype(bf16)  # [4, H]
        onehot4 = np.zeros((4, 4 * Bs), dtype=bf16)
        for g in range(4):
            onehot4[g, g * Bs : (g + 1) * Bs] = 1.0
        woutt = np.ascontiguousarray(2.0 * W_out.T).astype(bf16)
        in_maps = []
        for c in range(N_CORES):
            sh = 0.5 * h0[c * B_CORE : (c + 1) * B_CORE]  # h' = h/2
            in_maps.append(
                {
                    "h0t": np.ascontiguousarray(sh.T).astype(bf16),
                    "wt": wt,
                    "bstack4": bstack4,
                    "onehot4": onehot4,
                    "woutt": woutt,
                }
            )
    else:
        order = _gate_order(VARIANT)
        Wb = W.reshape(4, H, H)[order]
        bb = b.reshape(4, H)[order]
        wt = np.ascontiguousarray(Wb.reshape(4 * H, H).T)  # [H, 4H]
        bias_cols = np.ascontiguousarray(bb.T)  # [H, 4]
        woutt = np.ascontiguousarray(W_out.T)  # [H, O]

        in_maps = []
        for c in range(N_CORES):
            sh = h0[c * B_CORE : (c + 1) * B_CORE]  # [B_CORE, H]
            m = {
                "h0t": np.ascontiguousarray(sh.T),  # [H, B_CORE]
                "wt": wt,
                "bias": bias_cols,
                "woutt": woutt,
            }
            oh = _onehot_input(VARIANT)
            if oh is not None:
                m["onehot"] = oh
            in_maps.append(m)

    results = _get_runner(nc)(in_maps)

    out = np.empty((B_TOTAL, T, O), dtype=np.float32)
    for c in range(N_CORES):
        out[c * B_CORE : (c + 1) * B_CORE] = results[c]["preds"].reshape(B_CORE, T, O)
    out += b_out  # broadcast over [B, T, O]
    return out

